# Initial kernel scaffold
#
"""Trainium2 Bass kernel for nn_AttentionModel (spatio-temporal transformer).

B=4, N=64, T=192, H=256, FFN=1024, 3 layers, single-head attention over T
(temporal) then over N (spatial), post-LN, exact-erf GELU FFN.

Sharding across 8 NeuronCores:
  - temporal phase: shard N (8 n-rows/core, all b) -> 32 attention instances
  - spatial phase:  shard T (24 t/core, all b)     -> 96 attention instances
  - reshard between phases with an 8-core AllToAll (5 total)

On-device layouts (per core, 6144 tokens):
  - activations feature-major [H=2x128 part, 6144 free], float32r storage
    (TF32-rate matmuls when moving free dim >= 256)
  - temporal token order: tok  = (b*8 + n_local)*192 + t        (t contiguous)
  - spatial  token order: tok' = (b*64 + n)*24 + t_local        (t contiguous)
  - attention inner (Q/K/V/E/scores) in bf16, fp32 PSUM accumulate
  - softmax without max-subtraction (|scores| << 1 by construction);
    Z = col-sums of exp via ones-matmul, O normalized by 1/Z on evac
  - V bias folded into proj bias on host (proj(O/Z + bv) = proj(O/Z)+wp@bv+bp)
  - LayerNorm feature-major: sums via (1/H)-ones matmul broadcast into
    [128, W] PSUM, var = msq - mean^2, rstd = exp(-0.5*ln(var))
    (reference denom is std+1e-6; dropping eps is ~1e-5 relative)
"""
import os
import numpy as np
import ml_dtypes

B, N, T, D_IN, H, FH, LAYERS = 4, 64, 192, 2, 256, 1024, 3
NCORE = 8
TOK = B * N * T // NCORE            # 6144 tokens per core
NPC = N // NCORE                    # 8 n-rows per core (temporal)
TPC = T // NCORE                    # 24 t per core (spatial)
HC = H // 128                       # 2 feature chunks
INV_SCALE = 1.0 / float(H) ** 0.5   # 1/16
BF16 = ml_dtypes.bfloat16

LAST_EXEC_NS = None
_CACHE = {}


def _build_graph(trace_dummy=False):
    import concourse.bacc as bacc
    import concourse.mybir as mybir
    import concourse.tile as tile
    from concourse.alu_op_type import AluOpType

    dt = mybir.dt
    AF = mybir.ActivationFunctionType
    nc = bacc.Bacc("TRN2", target_bir_lowering=False, debug=False,
                   num_devices=NCORE)

    # ---------------- DRAM parameters ----------------
    ext = {}

    def din(name, shape, dty):
        ext[name] = nc.dram_tensor(name, list(shape), dty, kind="ExternalInput")
        return ext[name]

    din("x0", [D_IN, TOK], dt.float32r)
    din("wsT", [D_IN, H], dt.float32r)
    din("bs", [H, 1], dt.float32)
    for li in range(LAYERS):
        for a in ("t", "s"):
            for m in ("q", "k"):
                din(f"w{li}{a}{m}T", [H, H], dt.float32r)
                din(f"b{li}{a}{m}", [H, 1], dt.float32)
            din(f"w{li}{a}vT", [H, H], dt.float32r)
            din(f"w{li}{a}pT", [H, H], dt.bfloat16)
            din(f"b{li}{a}p", [H, 1], dt.float32)   # bp + wp@bv folded
        for ln in ("t", "s", "f"):
            din(f"g{li}{ln}", [H, 1], dt.float32)
            din(f"be{li}{ln}", [H, 1], dt.float32)
        din(f"w{li}f1T", [H, FH], dt.float32r)
        din(f"b{li}f1", [FH, 1], dt.float32)
        din(f"w{li}f2T", [FH, H], dt.bfloat16)
        din(f"b{li}f2", [H, 1], dt.float32)
    out_ext = nc.dram_tensor("out", [H, TOK], dt.float32, kind="ExternalOutput")

    RG = [list(range(NCORE))]

    with tile.TileContext(nc) as tc:
        with tc.tile_pool(name="stream", bufs=2) as stream_pool, \
             tc.tile_pool(name="wpool", bufs=2) as wpool, \
             tc.tile_pool(name="small", bufs=2) as small, \
             tc.tile_pool(name="blk", bufs=3) as blk, \
             tc.tile_pool(name="consts", bufs=1) as consts, \
             tc.tile_pool(name="ps", bufs=2, space="PSUM") as ps, \
             tc.tile_pool(name="dram", bufs=2, space="DRAM") as dram:

            ones_inv = consts.tile([128, 128], dt.float32r, tag="ones_inv")
            nc.vector.memset(ones_inv[:], 1.0 / H)
            ones_bf = consts.tile([128, 128], dt.bfloat16, tag="ones_bf")
            nc.vector.memset(ones_bf[:], 1.0)

            def load_w(name, kparts, fsz, dty, tag):
                """Weight [kparts*128 x fsz] as list of [128, fsz] tiles."""
                tiles = []
                for kc in range(kparts):
                    wt = wpool.tile([128, fsz], dty, tag=f"{tag}{kc}")
                    nc.sync.dma_start(out=wt[:], in_=ext[name][kc * 128:(kc + 1) * 128, :])
                    tiles.append(wt)
                return tiles

            def load_vec(name, tag, parts=HC):
                tiles = []
                for hc in range(parts):
                    vt = small.tile([128, 1], dt.float32, tag=f"{tag}{hc}")
                    nc.sync.dma_start(out=vt[:], in_=ext[name][hc * 128:(hc + 1) * 128, :])
                    tiles.append(vt)
                return tiles

            def new_stream(tag_prefix):
                return [stream_pool.tile([128, TOK], dt.float32r,
                                         tag=f"{tag_prefix}{hc}") for hc in range(HC)]

            def layernorm_block(S, W, gam, bet, X, x_aps):
                """LN over features: S = [HC][128,W] f32r tiles (contiguous),
                writes X[hc] at x_aps[hc] (AP of width W)."""
                ssq = [blk.tile([128, W], dt.float32r, tag=f"lnsq{hc}") for hc in range(HC)]
                for hc in range(HC):
                    nc.scalar.activation(ssq[hc][:], S[hc][:], AF.Square)
                mean_ps = ps.tile([128, W], dt.float32, tag="pstat")
                msq_ps = ps.tile([128, W], dt.float32, tag="pstat")
                for hc in range(HC):
                    nc.tensor.matmul(mean_ps[:], ones_inv[:], S[hc][:],
                                     start=(hc == 0), stop=(hc == HC - 1))
                for hc in range(HC):
                    nc.tensor.matmul(msq_ps[:], ones_inv[:], ssq[hc][:],
                                     start=(hc == 0), stop=(hc == HC - 1))
                m2 = blk.tile([128, W], dt.float32, tag="lnm2")
                nc.scalar.activation(m2[:], mean_ps[:], AF.Square)
                var = blk.tile([128, W], dt.float32, tag="lnvar")
                nc.vector.tensor_tensor(out=var[:], in0=msq_ps[:], in1=m2[:],
                                        op=AluOpType.subtract)
                lnv = blk.tile([128, W], dt.float32, tag="lnlnv")
                nc.scalar.activation(lnv[:], var[:], AF.Ln)
                rstd = blk.tile([128, W], dt.float32, tag="lnrstd")
                nc.scalar.activation(rstd[:], lnv[:], AF.Exp, scale=-0.5)
                for hc in range(HC):
                    d = blk.tile([128, W], dt.float32, tag=f"lnd{hc}")
                    nc.vector.tensor_tensor(out=d[:], in0=S[hc][:], in1=mean_ps[:],
                                            op=AluOpType.subtract)
                    xn = blk.tile([128, W], dt.float32, tag=f"lnxn{hc}")
                    nc.vector.tensor_tensor(out=xn[:], in0=d[:], in1=rstd[:],
                                            op=AluOpType.mult)
                    nc.scalar.activation(x_aps[hc], xn[:], AF.Copy,
                                         scale=gam[hc][:], bias=bet[hc][:])

            # ---------------- start linear: R = wsT.T @ x0 + bs ----------------
            x0_sb = consts.tile([D_IN, TOK], dt.float32r, tag="x0")
            nc.sync.dma_start(out=x0_sb[:], in_=ext["x0"][:, :])
            ws_sb = consts.tile([D_IN, H], dt.float32r, tag="ws")
            nc.sync.dma_start(out=ws_sb[:], in_=ext["wsT"][:, :])
            bs_t = load_vec("bs", "bs")
            R = new_stream("st")
            NCH = TOK // 512
            for hk in range(NCH):
                cs = slice(hk * 512, (hk + 1) * 512)
                for hc in range(HC):
                    p = ps.tile([128, 512], dt.float32, tag="plin")
                    nc.tensor.matmul(p[:], ws_sb[:, hc * 128:(hc + 1) * 128],
                                     x0_sb[:, cs], start=True, stop=True)
                    nc.scalar.activation(R[hc][:, cs], p[:], AF.Copy, bias=bs_t[hc][:])

            # ---------------- layers ----------------
            for li in range(LAYERS):
                # ===== temporal attention (32 instances of T=192) =====
                wq = load_w(f"w{li}tqT", HC, H, dt.float32r, "wq")
                wk = load_w(f"w{li}tkT", HC, H, dt.float32r, "wk")
                wv = load_w(f"w{li}tvT", HC, H, dt.float32r, "wv")
                wp = load_w(f"w{li}tpT", HC, H, dt.bfloat16, "wp")
                bq = load_vec(f"b{li}tq", "bq")
                bk = load_vec(f"b{li}tk", "bk")
                bp = load_vec(f"b{li}tp", "bp")
                gam = load_vec(f"g{li}t", "gam")
                bet = load_vec(f"be{li}t", "bet")
                X1 = new_stream("st")

                for bi in range(16):           # 2 instances per block
                    W = 384
                    cs = slice(bi * W, (bi + 1) * W)
                    # QKV linears (feature-major Q/K, token-major V)
                    qt = [blk.tile([128, W], dt.bfloat16, tag=f"qt{hc}") for hc in range(HC)]
                    kt = [blk.tile([128, W], dt.bfloat16, tag=f"kt{hc}") for hc in range(HC)]
                    for hc in range(HC):
                        p = ps.tile([128, W], dt.float32, tag="plin")
                        for kc in range(HC):
                            nc.tensor.matmul(p[:], wq[kc][:, hc * 128:(hc + 1) * 128],
                                             R[kc][:, cs], start=(kc == 0), stop=(kc == HC - 1))
                        nc.scalar.activation(qt[hc][:], p[:], AF.Copy, bias=bq[hc][:])
                        p2 = ps.tile([128, W], dt.float32, tag="plin")
                        for kc in range(HC):
                            nc.tensor.matmul(p2[:], wk[kc][:, hc * 128:(hc + 1) * 128],
                                             R[kc][:, cs], start=(kc == 0), stop=(kc == HC - 1))
                        nc.scalar.activation(kt[hc][:], p2[:], AF.Copy, bias=bk[hc][:])
                    # V token-major: 3 M-chunks of 128 tokens
                    vt = []
                    for mc in range(3):
                        pv = ps.tile([128, H], dt.float32, tag="pv")
                        if mc == 1:
                            lhs_ap = R_v_tail_ap = None
                        for kc in range(HC):
                            if mc == 0:
                                lhs = R[kc][:, bi * W:bi * W + 128]
                            elif mc == 2:
                                lhs = R[kc][:, bi * W + 192:bi * W + 320]
                            else:   # tails of both instances: cols 128..192, 320..384
                                lhs = R[kc][:, bi * W + 128:bi * W + 384].rearrange(
                                    "p (i t) -> p i t", i=2)[:, ::2, :].rearrange(
                                    "p i t -> p (i t)")
                            nc.tensor.matmul(pv[:], lhs, wv[kc][:],
                                             start=(kc == 0), stop=(kc == HC - 1))
                        v = blk.tile([128, H], dt.bfloat16, tag=f"vt{mc}")
                        nc.vector.tensor_copy(out=v[:], in_=pv[:])
                        vt.append(v)
                    # scores (transposed): St[m, l], m-chunks 128 + 64
                    st_a = ps.tile([128, W], dt.float32, tag="psta")
                    st_b = ps.tile([64, W], dt.float32, tag="pstb")
                    for ii in range(2):
                        lsl = slice(ii * 192, ii * 192 + 192)
                        for kc in range(HC):
                            nc.tensor.matmul(st_a[:, lsl],
                                             kt[kc][:, ii * 192:ii * 192 + 128],
                                             qt[kc][:, lsl], start=(kc == 0), stop=(kc == HC - 1))
                            nc.tensor.matmul(st_b[:, lsl],
                                             kt[kc][:, ii * 192 + 128:ii * 192 + 192],
                                             qt[kc][:, lsl], start=(kc == 0), stop=(kc == HC - 1))
                    e_a = blk.tile([128, W], dt.bfloat16, tag="ea")
                    e_b = blk.tile([64, W], dt.bfloat16, tag="eb")
                    nc.scalar.activation(e_a[:], st_a[:], AF.Exp, scale=INV_SCALE)
                    nc.scalar.activation(e_b[:], st_b[:], AF.Exp, scale=INV_SCALE)
                    # Z = column sums of exp
                    zb = ps.tile([128, W], dt.float32, tag="pz")
                    nc.tensor.matmul(zb[:], ones_bf[:], e_a[:], start=True, stop=False)
                    nc.tensor.matmul(zb[:], ones_bf[0:64, :], e_b[:], start=False, stop=True)
                    zinv = blk.tile([128, W], dt.float32, tag="zinv")
                    nc.vector.reciprocal(out=zinv[:], in_=zb[:])
                    # O = V.T-major accumulate: out[c, l]
                    o_ps = [ps.tile([128, W], dt.float32, tag=f"po{hc}") for hc in range(HC)]
                    for hc in range(HC):
                        csl = slice(hc * 128, (hc + 1) * 128)
                        for ii in range(2):
                            lsl = slice(ii * 192, ii * 192 + 192)
                            nc.tensor.matmul(o_ps[hc][:, lsl], vt[2 * ii][:, csl],
                                             e_a[:, lsl], start=True, stop=False)
                            nc.tensor.matmul(o_ps[hc][:, lsl],
                                             vt[1][ii * 64:(ii + 1) * 64, csl],
                                             e_b[:, lsl], start=False, stop=True)
                    o_sb = [blk.tile([128, W], dt.bfloat16, tag=f"ot{hc}") for hc in range(HC)]
                    for hc in range(HC):
                        nc.vector.tensor_tensor(out=o_sb[hc][:], in0=o_ps[hc][:],
                                                in1=zinv[:], op=AluOpType.mult)
                    # proj + residual -> S1; LN -> X1
                    S1 = [blk.tile([128, W], dt.float32r, tag=f"s1{hc}") for hc in range(HC)]
                    for hc in range(HC):
                        p = ps.tile([128, W], dt.float32, tag="plin")
                        for kc in range(HC):
                            nc.tensor.matmul(p[:], wp[kc][:, hc * 128:(hc + 1) * 128],
                                             o_sb[kc][:], start=(kc == 0), stop=(kc == HC - 1))
                        nc.vector.scalar_tensor_tensor(
                            out=S1[hc][:], in0=p[:], scalar=bp[hc][:], in1=R[hc][:, cs],
                            op0=AluOpType.add, op1=AluOpType.add)
                    layernorm_block(S1, W, gam, bet, X1,
                                    [X1[hc][:, cs] for hc in range(HC)])

                # ===== exchange 1: temporal -> spatial =====
                in_b = dram.tile([NCORE, H, B * NPC, TPC], dt.float32r, tag="exin")
                out_b = dram.tile([NCORE, H, B * NPC, TPC], dt.float32r, tag="exout")
                for j in range(NCORE):
                    for hc in range(HC):
                        src = X1[hc][:, :].rearrange("p (r t) -> p r t", t=T)[
                            :, :, j * TPC:(j + 1) * TPC]
                        nc.sync.dma_start(out=in_b[j, hc * 128:(hc + 1) * 128], in_=src)
                nc.gpsimd.collective_compute(
                    "AllToAll", mybir.AluOpType.bypass, replica_groups=RG,
                    ins=[in_b[:].opt()], outs=[out_b[:].opt()])
                Xs = new_stream("st")
                for j in range(NCORE):
                    for hc in range(HC):
                        # received [b][n8][t24] -> Xs cols (b*64+n)*24+t, n offset 8j
                        dst = Xs[hc][:, :].rearrange("p (b n t) -> p b n t", b=B, n=N)[
                            :, :, j * NPC:(j + 1) * NPC, :]
                        nc.sync.dma_start(
                            out=dst,
                            in_=out_b[j, hc * 128:(hc + 1) * 128].rearrange(
                                "p (b n) t -> p b n t", b=B))

                # ===== spatial attention (96 instances of N=64) =====
                wq = load_w(f"w{li}sqT", HC, H, dt.float32r, "wq")
                wk = load_w(f"w{li}skT", HC, H, dt.float32r, "wk")
                wv = load_w(f"w{li}svT", HC, H, dt.float32r, "wv")
                wp = load_w(f"w{li}spT", HC, H, dt.bfloat16, "wp")
                bq = load_vec(f"b{li}sq", "bq")
                bk = load_vec(f"b{li}sk", "bk")
                bp = load_vec(f"b{li}sp", "bp")
                gam = load_vec(f"g{li}s", "gam")
                bet = load_vec(f"be{li}s", "bet")
                X2 = new_stream("st")

                for bb in range(B):
                    for tg in range(TPC // 8):   # batches of 8 t-instances
                        t0 = tg * 8
                        # batch cols AP: [n:64 stride TPC][t:8 stride 1]
                        def bap(tensor_hc, hc):
                            return tensor_hc[hc][:, :].rearrange(
                                "p (b n t) -> p b n t", b=B, n=N)[:, bb, :, t0:t0 + 8]
                        qt = [blk.tile([128, 512], dt.bfloat16, tag=f"qt{hc}") for hc in range(HC)]
                        kt = [blk.tile([128, 512], dt.bfloat16, tag=f"kt{hc}") for hc in range(HC)]
                        for hc in range(HC):
                            p = ps.tile([128, 512], dt.float32, tag="plin")
                            for kc in range(HC):
                                nc.tensor.matmul(p[:], wq[kc][:, hc * 128:(hc + 1) * 128],
                                                 bap(Xs, kc), start=(kc == 0), stop=(kc == HC - 1))
                            nc.scalar.activation(qt[hc][:], p[:], AF.Copy, bias=bq[hc][:])
                            p2 = ps.tile([128, 512], dt.float32, tag="plin")
                            for kc in range(HC):
                                nc.tensor.matmul(p2[:], wk[kc][:, hc * 128:(hc + 1) * 128],
                                                 bap(Xs, kc), start=(kc == 0), stop=(kc == HC - 1))
                            nc.scalar.activation(kt[hc][:], p2[:], AF.Copy, bias=bk[hc][:])
                        # V token-major: 4 pairs of t-instances -> [128 = 2x64n, 256]
                        vt = []
                        for pr in range(4):
                            pv = ps.tile([128, H], dt.float32, tag="pv")
                            for kc in range(HC):
                                lhs = Xs[kc][:, :].rearrange(
                                    "p (b n t) -> p b t n", b=B, n=N)[
                                    :, bb, t0 + 2 * pr:t0 + 2 * pr + 2, :].rearrange(
                                    "p t n -> p (t n)")
                                nc.tensor.matmul(pv[:], lhs, wv[kc][:],
                                                 start=(kc == 0), stop=(kc == HC - 1))
                            v = blk.tile([128, H], dt.bfloat16, tag=f"vs{pr}")
                            nc.vector.tensor_copy(out=v[:], in_=pv[:])
                            vt.append(v)
                        # St per instance into [64, 512] psum
                        st = ps.tile([64, 512], dt.float32, tag="psta")
                        for dti in range(8):
                            lsl = slice(dti * 64, dti * 64 + 64)
                            for kc in range(HC):
                                karr = kt[kc][:, :].rearrange("p (n t) -> p t n", t=8)[:, dti, :]
                                qarr = qt[kc][:, :].rearrange("p (n t) -> p t n", t=8)[:, dti, :]
                                nc.tensor.matmul(st[:, lsl], karr, qarr,
                                                 start=(kc == 0), stop=(kc == HC - 1))
                        e = blk.tile([64, 512], dt.bfloat16, tag="ea")
                        nc.scalar.activation(e[:], st[:], AF.Exp, scale=INV_SCALE)
                        zb = ps.tile([128, 512], dt.float32, tag="pz")
                        nc.tensor.matmul(zb[:], ones_bf[0:64, :], e[:], start=True, stop=True)
                        zinv = blk.tile([128, 512], dt.float32, tag="zinv")
                        nc.vector.reciprocal(out=zinv[:], in_=zb[:])
                        o_ps = [ps.tile([128, 512], dt.float32, tag=f"po{hc}") for hc in range(HC)]
                        for hc in range(HC):
                            csl = slice(hc * 128, (hc + 1) * 128)
                            for dti in range(8):
                                lsl = slice(dti * 64, dti * 64 + 64)
                                nc.tensor.matmul(
                                    o_ps[hc][:, lsl],
                                    vt[dti // 2][(dti % 2) * 64:(dti % 2) * 64 + 64, csl],
                                    e[:, lsl], start=True, stop=True)
                        o_sb = [blk.tile([128, 512], dt.bfloat16, tag=f"ot{hc}") for hc in range(HC)]
                        for hc in range(HC):
                            nc.vector.tensor_tensor(out=o_sb[hc][:], in0=o_ps[hc][:],
                                                    in1=zinv[:], op=AluOpType.mult)
                        S2 = [blk.tile([128, 512], dt.float32r, tag=f"s1{hc}") for hc in range(HC)]
                        for hc in range(HC):
                            p = ps.tile([128, 512], dt.float32, tag="plin")
                            for kc in range(HC):
                                nc.tensor.matmul(p[:], wp[kc][:, hc * 128:(hc + 1) * 128],
                                                 o_sb[kc][:], start=(kc == 0), stop=(kc == HC - 1))
                            nc.vector.scalar_tensor_tensor(
                                out=S2[hc][:], in0=p[:], scalar=bp[hc][:], in1=bap(Xs, hc),
                                op0=AluOpType.add, op1=AluOpType.add)
                        # note: S2 cols are batch-local [n*8+dt]; X2 written strided
                        layernorm_block(S2, 512, gam, bet, X2,
                                        [bap(X2, hc) for hc in range(HC)])

                # ===== FFN + LN_f (spatial layout, contiguous 512-chunks) =====
                w1 = load_w(f"w{li}f1T", HC, FH, dt.float32r, "w1")
                w2 = load_w(f"w{li}f2T", FH // 128, H, dt.bfloat16, "w2")
                b1 = load_vec(f"b{li}f1", "b1", parts=FH // 128)
                b2 = load_vec(f"b{li}f2", "b2")
                gam = load_vec(f"g{li}f", "gam")
                bet = load_vec(f"be{li}f", "bet")
                X3 = new_stream("st")
                for hk in range(NCH):
                    cs = slice(hk * 512, (hk + 1) * 512)
                    mid = []
                    for fc in range(FH // 128):
                        p = ps.tile([128, 512], dt.float32, tag="plin")
                        for kc in range(HC):
                            nc.tensor.matmul(p[:], w1[kc][:, fc * 128:(fc + 1) * 128],
                                             X2[kc][:, cs], start=(kc == 0), stop=(kc == HC - 1))
                        m = blk.tile([128, 512], dt.bfloat16, tag=f"mid{fc}")
                        nc.scalar.activation(m[:], p[:], AF.Gelu, bias=b1[fc][:])
                        mid.append(m)
                    S3 = [blk.tile([128, 512], dt.float32r, tag=f"s1{hc}") for hc in range(HC)]
                    for hc in range(HC):
                        p = ps.tile([128, 512], dt.float32, tag="plin")
                        for kc in range(FH // 128):
                            nc.tensor.matmul(p[:], w2[kc][:, hc * 128:(hc + 1) * 128],
                                             mid[kc][:], start=(kc == 0), stop=(kc == FH // 128 - 1))
                        nc.vector.scalar_tensor_tensor(
                            out=S3[hc][:], in0=p[:], scalar=b2[hc][:], in1=X2[hc][:, cs],
                            op0=AluOpType.add, op1=AluOpType.add)
                    layernorm_block(S3, 512, gam, bet, X3,
                                    [X3[hc][:, cs] for hc in range(HC)])

                if li < LAYERS - 1:
                    # ===== exchange 2: spatial -> temporal =====
                    in_b = dram.tile([NCORE, H, B * NPC, TPC], dt.float32r, tag="exin")
                    out_b = dram.tile([NCORE, H, B * NPC, TPC], dt.float32r, tag="exout")
                    for j in range(NCORE):
                        for hc in range(HC):
                            src = X3[hc][:, :].rearrange(
                                "p (b n t) -> p b n t", b=B, n=N)[
                                :, :, j * NPC:(j + 1) * NPC, :].rearrange(
                                "p b n t -> p (b n) t")
                            nc.sync.dma_start(out=in_b[j, hc * 128:(hc + 1) * 128], in_=src)
                    nc.gpsimd.collective_compute(
                        "AllToAll", mybir.AluOpType.bypass, replica_groups=RG,
                        ins=[in_b[:].opt()], outs=[out_b[:].opt()])
                    R = new_stream("st")
                    for j in range(NCORE):
                        for hc in range(HC):
                            dst = R[hc][:, :].rearrange("p (r t) -> p r t", t=T)[
                                :, :, j * TPC:(j + 1) * TPC]
                            nc.sync.dma_start(out=dst, in_=out_b[j, hc * 128:(hc + 1) * 128])
                else:
                    # final output (spatial layout), f32r bits == f32
                    for hc in range(HC):
                        nc.sync.dma_start(
                            out=out_ext[hc * 128:(hc + 1) * 128, :],
                            in_=X3[hc][:, :].bitcast(dt.float32))

    nc.compile()
    return nc


def _prep_inputs(x, params):
    """Host-side: flatten params, pre-transpose, fold V bias into proj bias."""
    f32 = np.float32
    common = {}
    x = np.asarray(x, f32)

    def wT(p):
        return np.ascontiguousarray(np.asarray(p["w"], f32).T)

    def col(v):
        return np.ascontiguousarray(np.asarray(v, f32).reshape(-1, 1))

    common["wsT"] = wT(params["start"])
    common["bs"] = col(params["start"]["b"])
    for li, lp in enumerate(params["layers"]):
        for a, key in (("t", "t_attn"), ("s", "s_attn")):
            ap = lp[key]
            common[f"w{li}{a}qT"] = wT(ap["q"])
            common[f"b{li}{a}q"] = col(ap["q"]["b"])
            common[f"w{li}{a}kT"] = wT(ap["k"])
            common[f"b{li}{a}k"] = col(ap["k"]["b"])
            common[f"w{li}{a}vT"] = wT(ap["v"])
            common[f"w{li}{a}pT"] = wT(ap["p"]).astype(BF16)
            bp = np.asarray(ap["p"]["b"], f32) + \
                np.asarray(ap["p"]["w"], f32) @ np.asarray(ap["v"]["b"], f32)
            common[f"b{li}{a}p"] = col(bp)
        for ln, key in (("t", "ln_t"), ("s", "ln_s"), ("f", "ln_f")):
            common[f"g{li}{ln}"] = col(lp[key]["g"])
            common[f"be{li}{ln}"] = col(lp[key]["b"])
        common[f"w{li}f1T"] = wT(lp["ff1"])
        common[f"b{li}f1"] = col(lp["ff1"]["b"])
        common[f"w{li}f2T"] = wT(lp["ff2"]).astype(BF16)
        common[f"b{li}f2"] = col(lp["ff2"]["b"])

    in_maps = []
    for c in range(NCORE):
        m = dict(common)
        xs = x[:, c * NPC:(c + 1) * NPC, :, :]          # [B, 8, T, 2]
        m["x0"] = np.ascontiguousarray(
            xs.transpose(3, 0, 1, 2).reshape(D_IN, TOK))
        in_maps.append(m)
    return in_maps


def kernel(x, params):
    global LAST_EXEC_NS
    from concourse.bass_utils import run_bass_kernel_spmd

    if "nc" not in _CACHE:
        _CACHE["nc"] = _build_graph()
    nc = _CACHE["nc"]
    in_maps = _prep_inputs(x, params)
    trace = bool(int(os.environ.get("BASS_KERNEL_TRACE", "0")))
    res = run_bass_kernel_spmd(nc, in_maps, list(range(NCORE)), trace=trace)
    LAST_EXEC_NS = res.exec_time_ns

    xf = np.empty((B, N, T, H), np.float32)
    for c in range(NCORE):
        o = res.results[c]["out"]                        # [256, 6144]
        o = o.reshape(H, B, N, TPC).transpose(1, 2, 3, 0)  # [B, N, 24, H]
        xf[:, :, c * TPC:(c + 1) * TPC, :] = o
    return np.maximum(xf, 0.0), xf


# revision 13
# speedup vs baseline: 1.1441x; 1.1441x over previous
"""Trainium2 Bass kernel for nn_AttentionModel (spatio-temporal transformer).

B=4, N=64, T=192, H=256, FFN=1024, 3 layers, single-head attention over T
(temporal) then over N (spatial), post-LN, exact-erf GELU FFN.

Sharding across 8 NeuronCores:
  - temporal phase: shard N (8 n-rows/core, all b) -> 32 attention instances
  - spatial phase:  shard T (24 t/core, all b)     -> 96 attention instances
  - reshard between phases with an 8-core AllToAll (5 total)

Per-core on-device layouts (6144 tokens):
  - activations feature-major [H=2x128 part, 6144 free], float32r storage
    (TF32-rate matmuls when the moving free dim >= 256)
  - temporal token order: tok  = (b*8 + n_local)*192 + t        (t contiguous)
  - spatial  token order: tok' = (b*64 + n)*24 + t_local        (t contiguous)
  - attention inner (Q/K/V/E) in bf16 with fp32 PSUM accumulate
  - softmax without max-subtraction (|scores| << 1 by construction);
    Z = column sums of exp via ones-matmul, O scaled by 1/Z on evacuation
  - V bias folded into proj bias on host (proj(O/Z + bv) = proj(O/Z)+wp@bv+bp)
  - LayerNorm feature-major: sums via (1/H)-ones matmul broadcast into
    [128, W] PSUM, var = msq - mean^2, rstd = exp(-0.5*ln(var))
    (reference denom is std+1e-6; dropping eps is ~1e-5 relative)
"""
import os
import numpy as np
import ml_dtypes

B, N, T, D_IN, H, FH, LAYERS = 4, 64, 192, 2, 256, 1024, 3
NCORE = 8
TOK = B * N * T // NCORE            # 6144 tokens per core
NPC = N // NCORE                    # 8 n-rows per core (temporal)
TPC = T // NCORE                    # 24 t per core (spatial)
HC = H // 128                       # 2 feature chunks
FC = FH // 128                      # 8 ffn chunks
INV_SCALE = 1.0 / float(H) ** 0.5   # 1/16
NCH = TOK // 512                    # 12 token chunks for linears
BF16 = ml_dtypes.bfloat16

LAST_EXEC_NS = None
_CACHE = {}


def _build_graph(taps=False):
    import concourse.bacc as bacc
    import concourse.mybir as mybir
    import concourse.tile as tile
    from concourse.alu_op_type import AluOpType

    dt = mybir.dt
    AF = mybir.ActivationFunctionType
    nc = bacc.Bacc("TRN2", target_bir_lowering=False, debug=False,
                   num_devices=NCORE)

    ext = {}

    def din(name, shape, dty):
        ext[name] = nc.dram_tensor(name, list(shape), dty, kind="ExternalInput")

    din("x0", [D_IN, TOK], dt.float32r)
    din("cinv", [128, 128], dt.float32r)          # constant 1/H
    din("wsT", [D_IN, H], dt.float32r)
    din("bs", [H, 1], dt.float32)
    for li in range(LAYERS):
        for a in ("t", "s"):
            for m in ("q", "k"):
                din(f"w{li}{a}{m}T", [H, H], dt.float32r)
                din(f"b{li}{a}{m}", [H, 1], dt.float32)
            din(f"w{li}{a}vT", [H, H], dt.float32r)
            din(f"w{li}{a}pT", [H, H], dt.bfloat16)
            din(f"b{li}{a}p", [H, 1], dt.float32)   # bp + wp@bv folded
        for ln in ("t", "s", "f"):
            din(f"g{li}{ln}", [H, 1], dt.float32)
            din(f"be{li}{ln}", [H, 1], dt.float32)
        din(f"w{li}f1T", [H, FH], dt.float32r)
        din(f"b{li}f1", [FH, 1], dt.float32)
        din(f"w{li}f2T", [FH, H], dt.bfloat16)
        din(f"b{li}f2", [H, 1], dt.float32)
    out_ext = nc.dram_tensor("out", [H, TOK], dt.float32, kind="ExternalOutput")
    tap_ext = {}
    if taps:
        for tn in ("R0", "X1", "Xs", "X2", "X3", "R1"):
            tap_ext[tn] = nc.dram_tensor(f"tap_{tn}", [H, TOK], dt.float32,
                                         kind="ExternalOutput")

    RG = [list(range(NCORE))]

    with tile.TileContext(nc) as tc:
        with tc.tile_pool(name="stream", bufs=2) as stream_pool, \
             tc.tile_pool(name="wpool", bufs=1) as wpool, \
             tc.tile_pool(name="small", bufs=2) as small, \
             tc.tile_pool(name="blk", bufs=2) as blk, \
             tc.tile_pool(name="mid", bufs=1) as midp, \
             tc.tile_pool(name="consts", bufs=1) as consts, \
             tc.tile_pool(name="psA", bufs=3, space="PSUM") as psA, \
             tc.tile_pool(name="psV", bufs=1, space="PSUM") as psV, \
             tc.tile_pool(name="psO", bufs=2, space="PSUM") as psO, \
             tc.tile_pool(name="dram", bufs=2, space="DRAM") as dram:

            ones_inv = consts.tile([128, 128], dt.float32r, tag="ones_inv", name="ones_inv")
            nc.sync.dma_start(out=ones_inv[:], in_=ext["cinv"][:, :])
            ones_bf = consts.tile([128, 128], dt.bfloat16, tag="ones_bf", name="ones_bf")
            nc.vector.memset(ones_bf[:], 1.0)

            def tap(name, stream):
                if taps and name in tap_ext:
                    for hc in range(HC):
                        nc.sync.dma_start(
                            out=tap_ext[name][hc * 128:(hc + 1) * 128, :],
                            in_=stream[hc][:, :].bitcast(dt.float32))

            def load_w(name, kparts, fsz, dty, tag):
                tiles = []
                for kc in range(kparts):
                    wt = wpool.tile([128, fsz], dty, tag=f"{tag}{kc}")
                    nc.sync.dma_start(
                        out=wt[:], in_=ext[name][kc * 128:(kc + 1) * 128, :])
                    tiles.append(wt)
                return tiles

            def load_vec(name, tag, parts=HC):
                tiles = []
                for hc in range(parts):
                    vt = small.tile([128, 1], dt.float32, tag=f"{tag}{hc}", name=f"{tag}{hc}")
                    nc.sync.dma_start(
                        out=vt[:], in_=ext[name][hc * 128:(hc + 1) * 128, :])
                    tiles.append(vt)
                return tiles

            def new_stream():
                return [stream_pool.tile([128, TOK], dt.float32r, tag=f"st{hc}", name=f"st{hc}")
                        for hc in range(HC)]

            def layernorm_block(S, W, gam, bet, x_aps):
                """Feature-axis LN of S=[HC][128,W] f32r; writes x_aps[hc]."""
                ssq = [blk.tile([128, 512], dt.float32r, tag=f"lnsq{hc}", name=f"lnsq{hc}")
                       for hc in range(HC)]
                for hc in range(HC):
                    nc.scalar.activation(ssq[hc][:, 0:W], S[hc][:, 0:W], AF.Square)
                mean_ps = psA.tile([128, 512], dt.float32, tag="plin", name="plin")
                msq_ps = psA.tile([128, 512], dt.float32, tag="plin", name="plin")
                for hc in range(HC):
                    nc.tensor.matmul(mean_ps[:, 0:W], ones_inv[:], S[hc][:, 0:W],
                                     start=(hc == 0), stop=(hc == HC - 1))
                for hc in range(HC):
                    nc.tensor.matmul(msq_ps[:, 0:W], ones_inv[:], ssq[hc][:, 0:W],
                                     start=(hc == 0), stop=(hc == HC - 1))
                t = blk.tile([128, 512], dt.float32, tag="lnt", name="lnt")
                nc.scalar.activation(t[:, 0:W], mean_ps[:, 0:W], AF.Square)
                nc.vector.tensor_tensor(out=t[:, 0:W], in0=msq_ps[:, 0:W],
                                        in1=t[:, 0:W], op=AluOpType.subtract)
                nc.scalar.activation(t[:, 0:W], t[:, 0:W], AF.Ln)
                nc.scalar.activation(t[:, 0:W], t[:, 0:W], AF.Exp, scale=-0.5)
                for hc in range(HC):
                    d = blk.tile([128, 512], dt.float32, tag=f"lnd{hc}", name=f"lnd{hc}")
                    nc.vector.tensor_tensor(out=d[:, 0:W], in0=S[hc][:, 0:W],
                                            in1=mean_ps[:, 0:W],
                                            op=AluOpType.subtract)
                    nc.vector.tensor_tensor(out=d[:, 0:W], in0=d[:, 0:W],
                                            in1=t[:, 0:W], op=AluOpType.mult)
                    nc.scalar.activation(x_aps[hc], d[:, 0:W], AF.Identity,
                                         scale=gam[hc][:], bias=bet[hc][:])

            # ---------------- start linear: R = wsT.T @ x0 + bs ---------------
            ws_sb = consts.tile([D_IN, H], dt.float32r, tag="ws", name="ws")
            nc.sync.dma_start(out=ws_sb[:], in_=ext["wsT"][:, :])
            bs_t = load_vec("bs", "bs")
            R = new_stream()
            for hk in range(NCH):
                cs = slice(hk * 512, (hk + 1) * 512)
                x0c = blk.tile([D_IN, 512], dt.float32r, tag="x0c", name="x0c")
                nc.sync.dma_start(out=x0c[:], in_=ext["x0"][:, cs])
                for hc in range(HC):
                    p = psA.tile([128, 512], dt.float32, tag="plin", name="plin")
                    nc.tensor.matmul(p[:], ws_sb[:, hc * 128:(hc + 1) * 128],
                                     x0c[:], start=True, stop=True)
                    nc.scalar.activation(R[hc][:, cs], p[:], AF.Identity,
                                         bias=bs_t[hc][:])

            tap("R0", R)
            for li in range(LAYERS):
                # ============ temporal attention (32 instances, T=192) ========
                wq = load_w(f"w{li}tqT", HC, H, dt.float32r, "wq")
                wk = load_w(f"w{li}tkT", HC, H, dt.float32r, "wk")
                wv = load_w(f"w{li}tvT", HC, H, dt.float32r, "wv")
                wp = load_w(f"w{li}tpT", HC, H, dt.bfloat16, "wp")
                bq = load_vec(f"b{li}tq", "bq")
                bk = load_vec(f"b{li}tk", "bk")
                bp = load_vec(f"b{li}tp", "bp")
                gam = load_vec(f"g{li}t", "gam")
                bet = load_vec(f"be{li}t", "bet")
                X1 = new_stream()

                for bi in range(16):          # 2 instances per block, W=384
                    W = 384
                    c0 = bi * W
                    cs = slice(c0, c0 + W)
                    qt = [blk.tile([128, 512], dt.bfloat16, tag=f"qt{hc}", name=f"qt{hc}")
                          for hc in range(HC)]
                    kt = [blk.tile([128, 512], dt.bfloat16, tag=f"kt{hc}", name=f"kt{hc}")
                          for hc in range(HC)]
                    for hc in range(HC):
                        p = psA.tile([128, 512], dt.float32, tag="plin", name="plin")
                        for kc in range(HC):
                            nc.tensor.matmul(p[:, 0:W],
                                             wq[kc][:, hc * 128:(hc + 1) * 128],
                                             R[kc][:, cs],
                                             start=(kc == 0), stop=(kc == HC - 1))
                        nc.scalar.activation(qt[hc][:, 0:W], p[:, 0:W], AF.Identity,
                                             bias=bq[hc][:])
                        p2 = psA.tile([128, 512], dt.float32, tag="plin", name="plin")
                        for kc in range(HC):
                            nc.tensor.matmul(p2[:, 0:W],
                                             wk[kc][:, hc * 128:(hc + 1) * 128],
                                             R[kc][:, cs],
                                             start=(kc == 0), stop=(kc == HC - 1))
                        nc.scalar.activation(kt[hc][:, 0:W], p2[:, 0:W], AF.Identity,
                                             bias=bk[hc][:])
                    # V token-major, 3 M-chunks of 128 tokens:
                    #  vt0 = inst0 t0..127; vt1 = [inst0 t128..191 | inst1
                    #  t128..191]; vt2 = inst1 t0..127
                    vt = []
                    for mc in range(3):
                        pv = psV.tile([128, 512], dt.float32, tag="pv", name="pv")
                        if mc == 1:
                            # both instances' t=128..191 tails, packed along
                            # free: inst ii at cols [ii*256, ii*256+256)
                            nmm = 0
                            for ii in range(2):
                                for kc in range(HC):
                                    lhs = R[kc][:, c0 + 128 + ii * 192:
                                                c0 + 192 + ii * 192]
                                    nc.tensor.matmul(
                                        pv[0:64, ii * 256:(ii + 1) * 256],
                                        lhs, wv[kc][:],
                                        start=(nmm == 0), stop=(nmm == 3))
                                    nmm += 1
                            v = blk.tile([64, 512], dt.bfloat16, tag="vtt",
                                         name="vtt")
                            nc.vector.tensor_copy(out=v[:], in_=pv[0:64, :])
                        else:
                            for kc in range(HC):
                                lhs = (R[kc][:, c0:c0 + 128] if mc == 0
                                       else R[kc][:, c0 + 192:c0 + 320])
                                nc.tensor.matmul(pv[:, 0:H], lhs, wv[kc][:],
                                                 start=(kc == 0), stop=(kc == HC - 1))
                            v = blk.tile([128, H], dt.bfloat16, tag=f"vt{mc}",
                                         name=f"vt{mc}")
                            nc.vector.tensor_copy(out=v[:], in_=pv[:, 0:H])
                        vt.append(v)
                    # scores transposed St[m, l]; m chunks 128 ("a") + 64 ("b")
                    st_a = psO.tile([128, 512], dt.float32, tag="po0", name="po0")
                    st_b = psO.tile([64, 512], dt.float32, tag="po1", name="po1")
                    nmm = 0
                    for ii in range(2):
                        lsl = slice(ii * 192, ii * 192 + 192)
                        for kc in range(HC):
                            nc.tensor.matmul(st_a[:, lsl],
                                             kt[kc][:, ii * 192:ii * 192 + 128],
                                             qt[kc][:, lsl],
                                             start=(nmm == 0), stop=(nmm == 3))
                            nc.tensor.matmul(st_b[:, lsl],
                                             kt[kc][:, ii * 192 + 128:ii * 192 + 192],
                                             qt[kc][:, lsl],
                                             start=(nmm == 0), stop=(nmm == 3))
                            nmm += 1
                    e_a = blk.tile([128, 512], dt.bfloat16, tag="ea", name="ea")
                    e_b = blk.tile([64, 512], dt.bfloat16, tag="eb", name="eb")
                    nc.scalar.activation(e_a[:, 0:W], st_a[:, 0:W], AF.Exp,
                                         scale=INV_SCALE)
                    nc.scalar.activation(e_b[:, 0:W], st_b[:, 0:W], AF.Exp,
                                         scale=INV_SCALE)
                    zb = psV.tile([128, 512], dt.float32, tag="pv", name="pv")
                    nc.tensor.matmul(zb[:, 0:W], ones_bf[:], e_a[:, 0:W],
                                     start=True, stop=False)
                    nc.tensor.matmul(zb[:, 0:W], ones_bf[0:64, :], e_b[:, 0:W],
                                     start=False, stop=True)
                    zinv = blk.tile([128, 512], dt.float32, tag="zinv", name="zinv")
                    nc.vector.reciprocal(out=zinv[:, 0:W], in_=zb[:, 0:W])
                    o_ps = [psO.tile([128, 512], dt.float32, tag=f"po{hc}", name=f"po{hc}")
                            for hc in range(HC)]
                    for hc in range(HC):
                        csl = slice(hc * 128, (hc + 1) * 128)
                        nmm = 0
                        for ii in range(2):
                            lsl = slice(ii * 192, ii * 192 + 192)
                            nc.tensor.matmul(o_ps[hc][:, lsl],
                                             vt[2 * ii][:, csl], e_a[:, lsl],
                                             start=(nmm == 0), stop=False)
                            nc.tensor.matmul(
                                o_ps[hc][:, lsl],
                                vt[1][0:64, ii * 256 + hc * 128:
                                      ii * 256 + hc * 128 + 128],
                                e_b[:, lsl],
                                start=False, stop=(nmm == 2))
                            nmm += 2
                    o_sb = [blk.tile([128, 512], dt.bfloat16, tag=f"ot{hc}", name=f"ot{hc}")
                            for hc in range(HC)]
                    for hc in range(HC):
                        nc.vector.tensor_tensor(out=o_sb[hc][:, 0:W],
                                                in0=o_ps[hc][:, 0:W],
                                                in1=zinv[:, 0:W],
                                                op=AluOpType.mult)
                    S1 = [blk.tile([128, 512], dt.float32r, tag=f"s1{hc}", name=f"s1{hc}")
                          for hc in range(HC)]
                    for hc in range(HC):
                        p = psA.tile([128, 512], dt.float32, tag="plin", name="plin")
                        for kc in range(HC):
                            nc.tensor.matmul(p[:, 0:W],
                                             wp[kc][:, hc * 128:(hc + 1) * 128],
                                             o_sb[kc][:, 0:W],
                                             start=(kc == 0), stop=(kc == HC - 1))
                        nc.vector.scalar_tensor_tensor(
                            out=S1[hc][:, 0:W], in0=p[:, 0:W], scalar=bp[hc][:],
                            in1=R[hc][:, cs], op0=AluOpType.add, op1=AluOpType.add)
                    layernorm_block(S1, W, gam, bet,
                                    [X1[hc][:, cs] for hc in range(HC)])

                if li == 0:
                    tap("X1", X1)
                # ============ exchange 1: temporal -> spatial =================
                in_b = dram.tile([NCORE, H, B * NPC, TPC], dt.float32, tag="exin", name="exin")
                out_b = dram.tile([NCORE, H, B * NPC, TPC], dt.float32, tag="exout", name="exout")
                for j in range(NCORE):
                    for hc in range(HC):
                        src = X1[hc][:, :].rearrange("p (r t) -> p r t", t=T)[
                            :, :, j * TPC:(j + 1) * TPC]
                        nc.gpsimd.dma_start(
                            out=in_b[j, hc * 128:(hc + 1) * 128], in_=src)
                nc.gpsimd.collective_compute(
                    "AllToAll", mybir.AluOpType.bypass, replica_groups=RG,
                    ins=[in_b[:].opt()], outs=[out_b[:].opt()])
                Xs = new_stream()
                for j in range(NCORE):
                    for hc in range(HC):
                        dst = Xs[hc][:, :].rearrange(
                            "p (b n t) -> p b n t", b=B, n=N)[
                            :, :, j * NPC:(j + 1) * NPC, :]
                        nc.gpsimd.dma_start(
                            out=dst,
                            in_=out_b[j, hc * 128:(hc + 1) * 128].rearrange(
                                "p (b n) t -> p b n t", b=B))

                if li == 0:
                    tap("Xs", Xs)
                # ============ spatial attention (96 instances, N=64) ==========
                wq = load_w(f"w{li}sqT", HC, H, dt.float32r, "wq")
                wk = load_w(f"w{li}skT", HC, H, dt.float32r, "wk")
                wv = load_w(f"w{li}svT", HC, H, dt.float32r, "wv")
                wp = load_w(f"w{li}spT", HC, H, dt.bfloat16, "wp")
                bq = load_vec(f"b{li}sq", "bq")
                bk = load_vec(f"b{li}sk", "bk")
                bp = load_vec(f"b{li}sp", "bp")
                gam = load_vec(f"g{li}s", "gam")
                bet = load_vec(f"be{li}s", "bet")
                X2 = new_stream()

                for bb in range(B):
                    # Q/K for all of b's 1536 tokens, 3 contiguous chunks
                    qt = [blk.tile([128, 3 * 512], dt.bfloat16, tag=f"qts{hc}",
                                   name=f"qts{hc}", bufs=1) for hc in range(HC)]
                    kt = [blk.tile([128, 3 * 512], dt.bfloat16, tag=f"kts{hc}",
                                   name=f"kts{hc}", bufs=1) for hc in range(HC)]
                    for ck in range(3):
                        cs = slice(bb * 1536 + ck * 512, bb * 1536 + ck * 512 + 512)
                        lsl = slice(ck * 512, ck * 512 + 512)
                        for hc in range(HC):
                            p = psA.tile([128, 512], dt.float32, tag="plin", name="plin")
                            for kc in range(HC):
                                nc.tensor.matmul(p[:], wq[kc][:, hc * 128:(hc + 1) * 128],
                                                 Xs[kc][:, cs],
                                                 start=(kc == 0), stop=(kc == HC - 1))
                            nc.scalar.activation(qt[hc][:, lsl], p[:], AF.Identity,
                                                 bias=bq[hc][:])
                            p2 = psA.tile([128, 512], dt.float32, tag="plin", name="plin")
                            for kc in range(HC):
                                nc.tensor.matmul(p2[:], wk[kc][:, hc * 128:(hc + 1) * 128],
                                                 Xs[kc][:, cs],
                                                 start=(kc == 0), stop=(kc == HC - 1))
                            nc.scalar.activation(kt[hc][:, lsl], p2[:], AF.Identity,
                                                 bias=bk[hc][:])

                    def iap(stream, hc, t, bb=bb):
                        # one instance's 64 n-columns at fixed (b, t): stride TPC
                        return stream[hc][:, :].rearrange(
                            "p (c t) -> p t c", t=TPC)[:, t, bb * 64:(bb + 1) * 64]

                    for tg in range(TPC // 8):    # batches of 8 t-instances
                        t0 = tg * 8

                        def bap(stream, hc, bb=bb, t0=t0):
                            return stream[hc][:, :].rearrange(
                                "p (b n t) -> p b n t", b=B, n=N)[
                                :, bb, :, t0:t0 + 8]
                        # V token-major: per-instance M=64 matmuls, pairs share
                        # a [128, 256] psum/sbuf tile (inst t0+2p at rows 0:64)
                        vt = []
                        for pr in range(4):
                            pv = psV.tile([128, 512], dt.float32, tag="pv", name="pv")
                            nmm = 0
                            for ii in range(2):
                                for kc in range(HC):
                                    nc.tensor.matmul(
                                        pv[0:64, ii * 256:(ii + 1) * 256],
                                        iap(Xs, kc, t0 + 2 * pr + ii), wv[kc][:],
                                        start=(nmm == 0), stop=(nmm == 3))
                                    nmm += 1
                            v = blk.tile([64, 512], dt.bfloat16, tag=f"vt{pr}", name=f"vt{pr}")
                            nc.vector.tensor_copy(out=v[:], in_=pv[0:64, :])
                            vt.append(v)
                        st = psO.tile([64, 512], dt.float32, tag="po0", name="po0")
                        nmm = 0
                        for dti in range(8):
                            lsl = slice(dti * 64, dti * 64 + 64)
                            kq_col = (t0 + dti)
                            for kc in range(HC):
                                karr = kt[kc][:, :].rearrange(
                                    "p (c t) -> p t c", t=TPC)[:, kq_col, :]
                                qarr = qt[kc][:, :].rearrange(
                                    "p (c t) -> p t c", t=TPC)[:, kq_col, :]
                                nc.tensor.matmul(st[:, lsl], karr, qarr,
                                                 start=(nmm == 0), stop=(nmm == 15))
                                nmm += 1
                        e = blk.tile([64, 512], dt.bfloat16, tag="ea", name="ea")
                        nc.scalar.activation(e[:], st[:], AF.Exp, scale=INV_SCALE)
                        zb = psV.tile([128, 512], dt.float32, tag="pv", name="pv")
                        nc.tensor.matmul(zb[:], ones_bf[0:64, :], e[:],
                                         start=True, stop=True)
                        zinv = blk.tile([128, 512], dt.float32, tag="zinv", name="zinv")
                        nc.vector.reciprocal(out=zinv[:], in_=zb[:])
                        o_ps = [psO.tile([128, 512], dt.float32, tag=f"po{hc}", name=f"po{hc}")
                                for hc in range(HC)]
                        for hc in range(HC):
                            csl = slice(hc * 128, (hc + 1) * 128)
                            for dti in range(8):
                                lsl = slice(dti * 64, dti * 64 + 64)
                                nc.tensor.matmul(
                                    o_ps[hc][:, lsl],
                                    vt[dti // 2][0:64, (dti % 2) * 256 + hc * 128:
                                                 (dti % 2) * 256 + hc * 128 + 128],
                                    e[:, lsl],
                                    start=(dti == 0), stop=(dti == 7))
                        o_sb = [blk.tile([128, 512], dt.bfloat16, tag=f"ot{hc}", name=f"ot{hc}")
                                for hc in range(HC)]
                        for hc in range(HC):
                            nc.vector.tensor_tensor(out=o_sb[hc][:], in0=o_ps[hc][:],
                                                    in1=zinv[:], op=AluOpType.mult)
                        S2 = [blk.tile([128, 512], dt.float32r, tag=f"s1{hc}", name=f"s1{hc}")
                              for hc in range(HC)]
                        for hc in range(HC):
                            p = psA.tile([128, 512], dt.float32, tag="plin", name="plin")
                            for kc in range(HC):
                                nc.tensor.matmul(p[:], wp[kc][:, hc * 128:(hc + 1) * 128],
                                                 o_sb[kc][:],
                                                 start=(kc == 0), stop=(kc == HC - 1))
                            nc.vector.scalar_tensor_tensor(
                                out=S2[hc][:], in0=p[:], scalar=bp[hc][:],
                                in1=bap(Xs, hc), op0=AluOpType.add, op1=AluOpType.add)
                        layernorm_block(S2, 512, gam, bet,
                                        [bap(X2, hc) for hc in range(HC)])

                if li == 0:
                    tap("X2", X2)
                # ============ FFN + LN_f (spatial layout, 512-chunks) =========
                w1 = load_w(f"w{li}f1T", HC, FH, dt.float32r, "w1")
                w2 = load_w(f"w{li}f2T", FC, H, dt.bfloat16, "w2")
                b1 = load_vec(f"b{li}f1", "b1", parts=FC)
                b2 = load_vec(f"b{li}f2", "b2")
                gam = load_vec(f"g{li}f", "gam")
                bet = load_vec(f"be{li}f", "bet")
                X3 = new_stream()
                for hk in range(NCH):
                    cs = slice(hk * 512, (hk + 1) * 512)
                    mid = []
                    for fc in range(FC):
                        p = psA.tile([128, 512], dt.float32, tag="plin", name="plin")
                        for kc in range(HC):
                            nc.tensor.matmul(p[:], w1[kc][:, fc * 128:(fc + 1) * 128],
                                             X2[kc][:, cs],
                                             start=(kc == 0), stop=(kc == HC - 1))
                        m = midp.tile([128, 512], dt.bfloat16, tag=f"mid{fc}", name=f"mid{fc}")
                        nc.scalar.activation(m[:], p[:], AF.Gelu, bias=b1[fc][:])
                        mid.append(m)
                    S3 = [blk.tile([128, 512], dt.float32r, tag=f"s1{hc}", name=f"s1{hc}")
                          for hc in range(HC)]
                    for hc in range(HC):
                        p = psA.tile([128, 512], dt.float32, tag="plin", name="plin")
                        for kc in range(FC):
                            nc.tensor.matmul(p[:], w2[kc][:, hc * 128:(hc + 1) * 128],
                                             mid[kc][:],
                                             start=(kc == 0), stop=(kc == FC - 1))
                        nc.vector.scalar_tensor_tensor(
                            out=S3[hc][:], in0=p[:], scalar=b2[hc][:],
                            in1=X2[hc][:, cs], op0=AluOpType.add, op1=AluOpType.add)
                    layernorm_block(S3, 512, gam, bet,
                                    [X3[hc][:, cs] for hc in range(HC)])

                if li == 0:
                    tap("X3", X3)
                if li < LAYERS - 1:
                    # ============ exchange 2: spatial -> temporal =============
                    in_b = dram.tile([NCORE, H, B * NPC, TPC], dt.float32,
                                     tag="exin")
                    out_b = dram.tile([NCORE, H, B * NPC, TPC], dt.float32,
                                      tag="exout")
                    for j in range(NCORE):
                        for hc in range(HC):
                            src = X3[hc][:, :].rearrange(
                                "p (b n t) -> p b n t", b=B, n=N)[
                                :, :, j * NPC:(j + 1) * NPC, :]
                            nc.gpsimd.dma_start(
                                out=in_b[j, hc * 128:(hc + 1) * 128].rearrange(
                                    "p (b n) t -> p b n t", b=B),
                                in_=src)
                    nc.gpsimd.collective_compute(
                        "AllToAll", mybir.AluOpType.bypass, replica_groups=RG,
                        ins=[in_b[:].opt()], outs=[out_b[:].opt()])
                    R = new_stream()
                    for j in range(NCORE):
                        for hc in range(HC):
                            dst = R[hc][:, :].rearrange("p (r t) -> p r t", t=T)[
                                :, :, j * TPC:(j + 1) * TPC]
                            nc.gpsimd.dma_start(
                                out=dst, in_=out_b[j, hc * 128:(hc + 1) * 128])
                    if li == 0:
                        tap("R1", R)
                else:
                    for hc in range(HC):
                        nc.sync.dma_start(
                            out=out_ext[hc * 128:(hc + 1) * 128, :],
                            in_=X3[hc][:, :].bitcast(dt.float32))

    nc.compile()
    return nc


def _prep_inputs(x, params):
    """Host-side: flatten params, pre-transpose, fold V bias into proj bias."""
    f32 = np.float32
    common = {}
    x = np.asarray(x, f32)

    def wT(p):
        return np.ascontiguousarray(np.asarray(p["w"], f32).T)

    def col(v):
        return np.ascontiguousarray(np.asarray(v, f32).reshape(-1, 1))

    common["cinv"] = np.full((128, 128), 1.0 / H, f32)
    common["wsT"] = wT(params["start"])
    common["bs"] = col(params["start"]["b"])
    for li, lp in enumerate(params["layers"]):
        for a, key in (("t", "t_attn"), ("s", "s_attn")):
            ap = lp[key]
            common[f"w{li}{a}qT"] = wT(ap["q"])
            common[f"b{li}{a}q"] = col(ap["q"]["b"])
            common[f"w{li}{a}kT"] = wT(ap["k"])
            common[f"b{li}{a}k"] = col(ap["k"]["b"])
            common[f"w{li}{a}vT"] = wT(ap["v"])
            common[f"w{li}{a}pT"] = wT(ap["p"]).astype(BF16)
            bp = np.asarray(ap["p"]["b"], f32) + \
                np.asarray(ap["p"]["w"], f32) @ np.asarray(ap["v"]["b"], f32)
            common[f"b{li}{a}p"] = col(bp)
        for ln, key in (("t", "ln_t"), ("s", "ln_s"), ("f", "ln_f")):
            common[f"g{li}{ln}"] = col(lp[key]["g"])
            common[f"be{li}{ln}"] = col(lp[key]["b"])
        common[f"w{li}f1T"] = wT(lp["ff1"])
        common[f"b{li}f1"] = col(lp["ff1"]["b"])
        common[f"w{li}f2T"] = wT(lp["ff2"]).astype(BF16)
        common[f"b{li}f2"] = col(lp["ff2"]["b"])

    in_maps = []
    for c in range(NCORE):
        m = dict(common)
        xs = x[:, c * NPC:(c + 1) * NPC, :, :]          # [B, 8, T, 2]
        m["x0"] = np.ascontiguousarray(
            xs.transpose(3, 0, 1, 2).reshape(D_IN, TOK))
        in_maps.append(m)
    return in_maps


def kernel(x, params):
    global LAST_EXEC_NS
    from concourse.bass_utils import run_bass_kernel_spmd

    if "nc" not in _CACHE:
        _CACHE["nc"] = _build_graph()
    nc = _CACHE["nc"]
    in_maps = _prep_inputs(x, params)
    trace = bool(int(os.environ.get("BASS_KERNEL_TRACE", "0")))
    res = run_bass_kernel_spmd(nc, in_maps, list(range(NCORE)), trace=trace)
    LAST_EXEC_NS = res.exec_time_ns

    xf = np.empty((B, N, T, H), np.float32)
    for c in range(NCORE):
        o = res.results[c]["out"]                          # [256, 6144]
        o = o.reshape(H, B, N, TPC).transpose(1, 2, 3, 0)  # [B, N, 24, H]
        xf[:, :, c * TPC:(c + 1) * TPC, :] = o
    return np.maximum(xf, 0.0), xf
